# revision 3
# baseline (speedup 1.0000x reference)
"""Trainium2 Bass kernel for nn_NeighborhoodAttention (GNN message passing).

v2 strategy (single SPMD program, no collectives):
  - Host: sort edges by dst, pad nodes 50000->50176 = 392 tiles of 128; core c
    owns 49 contiguous node tiles; per node tile the edge list is padded to
    128-edge blocks; descending-count tile order makes the per-position block
    count shared across cores; total blocks padded to full 32-block slabs.
  - Inputs staged feature-major: XK fp8e4 [81,2,E] for the k-path (DoubleRow),
    XA/XB fp8e3 (e3m4) for the v-path moving operands (stationary v weights
    stay bf16), OH fp8e4 host-built onehot [128, NBLK, 128] for the scatter.
  - L0 per 512-edge chunk: k fp8 DR matmul (rows 0-80) || v-XB (rows 96-127),
    then v-XA (full array); both land in one [128,2,512] PSUM pair evacuated
    by a single relu(2x) activation into an interleaved e3m4 hkv tile
    (scale 2 compensated via exp scale and oW0/2).
  - Per-block em matmuls with fp8 stationaries (FWL): scores (hk-slice x AQ8
    bf16, N=8), v (hv-slice x MW1v bf16, N=128), scatter (onehot x exvs bf16,
    N=136). exp per half-slab writes ex directly into exvs[:,:,128:136]; DVE
    multiplies ex into v per 8-block octet.
  - Node epilogue: r1=1/den, g=S*r1 (f32), PE transpose, relu folded into the
    post-transpose activation, 2-layer out MLP, feature-major OUT.
  - PSUM: L0 pair ring bufs=1 (2 banks), scores half-slab (1), v octet (2),
    S ring bufs=2 (2), epilogue tp/h0/o2 packed f32 tile (1) = 8 banks.
"""
import os
import sys
from contextlib import ExitStack

import ml_dtypes
import numpy as np

sys.path.insert(0, "/opt/trn_rl_repo")

import concourse.bass as bass
import concourse.tile as tile
from concourse import mybir
from concourse.bass_utils import run_bass_kernel_spmd
from concourse.vector_clock import ScopedClock


def _patched_drain_and_barrier(self, tick_clock, wait_clock):
    # Workaround: walrus CoreV3 setupSyncWait rejects >couple sem-waits on a
    # CTRL-class (drain) instruction. Spread the tail-drain waits across
    # preceding sync-engine nops (1 wait each) and leave the drain clean.
    nc = self.nc
    nop0 = nc.sync.nop(hint="tile_drain_waits", nofuse=True)
    wait_clock.add_sem_waits(nop0.ins, ScopedClock({None: tick_clock.global_clock}))
    si = nop0.ins.sync_info
    waits = list(si.on_wait) if si is not None and si.on_wait else []
    if len(waits) > 1:
        si.on_wait = waits[:1]
        for w in waits[1:]:
            ni = nc.sync.nop(hint="tile_drain_waits", nofuse=True)
            nsi = ni.ins.sync_info
            if nsi is None:
                ni.ins.sync_info = mybir.SyncInfo(on_wait=[w], on_update=[])
            else:
                nsi.on_wait = [w]
    nc.sync.drain()
    nc.all_engine_barrier()
    popped = nc._tile_sem_poison_stack.pop()
    assert popped is self._sem_poison
    nc.clear_and_free_semaphores(list(self.sems.allocated().values()))
    nc.all_engine_barrier()


tile.TileContext._drain_and_barrier = _patched_drain_and_barrier


def _split_excess_waits(nc, max_waits=1):
    """Walrus CoreV3 setupSyncWait rejects instructions with more than one
    sem-wait. Hoist excess waits onto same-engine nops inserted just before
    the offending instruction (program order per engine is the bb order)."""
    f = nc.m.functions[0]
    offenders = {}  # name -> list of hoisted-nop Instructions
    created = set()
    for bb in f.blocks:
        for inst in bb.instructions:
            si = inst.sync_info
            if si is None or not si.on_wait or len(si.on_wait) <= max_waits:
                continue
            w = list(si.on_wait)
            nops = []
            for wt in w[:-max_waits]:
                bi = nc.engines[inst.engine].nop(nofuse=True)
                nsi = bi.ins.sync_info
                if nsi is None:
                    bi.ins.sync_info = mybir.SyncInfo(on_wait=[wt], on_update=[])
                else:
                    nsi.on_wait = [wt]
                nops.append(bi.ins)
                created.add(bi.ins.name)
            si.on_wait = w[-max_waits:]
            offenders[inst.name] = nops
    if not offenders:
        return
    for bb in f.blocks:
        insts = list(bb.instructions)
        out = []
        changed = False
        for inst in insts:
            if inst.name in created:
                changed = True
                continue
            if inst.name in offenders:
                out.extend(offenders[inst.name])
                changed = True
            out.append(inst)
        if changed:
            bb.instructions = out

# problem constants (hardcoded per contract)
N, E = 50000, 800000
SRCF, DSTF, EDGEF = 64, 64, 32
D, H, DH = 128, 8, 16
SCALE = 1.0 / np.sqrt(np.float32(DH))
NCORES = 8
P = 128
NT_TOTAL = 392
TPC = NT_TOTAL // NCORES        # 49 node tiles per core
NPC = TPC * P                   # 6272 nodes per core
SLAB_BLOCKS = 32                # 32 blocks = 4096 edges per slab
SK = 2.0                        # hk evac scale (e3m4 range headroom)
SV = 2.0                        # hv evac scale
F32 = mybir.dt.float32
BF16 = mybir.dt.bfloat16
F8E4 = mybir.dt.float8e4
F8E3 = mybir.dt.float8e3

EVAC_DVE_POS = (1, 5)           # chunk positions whose L0 evac runs on DVE


# ----------------------------------------------------------------- host prep
def _prep(inputs):
    x_src = np.asarray(inputs["x_src"], np.float32)
    x_dst = np.asarray(inputs["x_dst"], np.float32)
    edge_attr = np.asarray(inputs["edge_attr"], np.float32)
    ei = np.asarray(inputs["edge_index"])
    src = ei[0].astype(np.int64)
    dst = ei[1].astype(np.int64)

    perm = np.argsort(dst, kind="stable")
    src_s, dst_s = src[perm], dst[perm]
    ea_s = edge_attr[perm]
    tile_counts = np.bincount(dst_s // P, minlength=NT_TOTAL)
    tile_starts = np.zeros(NT_TOTAL + 1, np.int64)
    np.cumsum(tile_counts, out=tile_starts[1:])

    node_deg = np.bincount(dst_s, minlength=N)
    need_eps = bool((node_deg == 0).any())

    orders = np.zeros((NCORES, TPC), np.int64)
    sorted_counts = np.zeros((NCORES, TPC), np.int64)
    for c in range(NCORES):
        tiles = np.arange(c * TPC, (c + 1) * TPC)
        o = np.argsort(-tile_counts[tiles], kind="stable")
        orders[c] = tiles[o]
        sorted_counts[c] = tile_counts[orders[c]]
    B = np.maximum(np.ceil(sorted_counts.max(axis=0) / P).astype(np.int64), 1)
    pad_blocks = (-int(B.sum())) % SLAB_BLOCKS
    B[-1] += pad_blocks
    Bcum = np.zeros(TPC + 1, np.int64)
    np.cumsum(B, out=Bcum[1:])
    NBLK = int(B.sum())
    EPAD = NBLK * P

    slot = np.full((NCORES, EPAD), -1, np.int64)
    dstloc = np.full((NCORES, EPAD), -1, np.int64)
    for c in range(NCORES):
        for j in range(TPC):
            t = orders[c, j]
            s0, cnt = int(tile_starts[t]), int(tile_counts[t])
            pos = int(Bcum[j]) * P
            slot[c, pos:pos + cnt] = np.arange(s0, s0 + cnt)
            dstloc[c, pos:pos + cnt] = dst_s[s0:s0 + cnt] - t * P

    real = slot >= 0
    slot_c = np.where(real, slot, 0)
    bf = ml_dtypes.bfloat16
    f8e4 = ml_dtypes.float8_e4m3
    f8e3 = ml_dtypes.float8_e3m4
    XA = np.zeros((NCORES, 128, EPAD), f8e3)
    XB = np.zeros((NCORES, 32, EPAD), f8e3)
    XK = np.zeros((NCORES, 81, 2, EPAD), f8e4)
    for c in range(NCORES):
        r = real[c]
        xs = np.where(r, x_src[src_s[slot_c[c]]].T, 0)
        xd = np.where(r, x_dst[dst_s[slot_c[c]]].T, 0)
        eb = np.where(r, ea_s[slot_c[c]].T, 0)
        XA[c, :64] = xs
        XA[c, 64:] = xd
        XB[c] = eb
        xfull = np.zeros((162, EPAD), np.float32)
        xfull[:64] = xs
        xfull[64:128] = xd
        xfull[128:160] = eb
        XK[c] = xfull.reshape(81, 2, EPAD).astype(f8e4)
    # onehot, exact in fp8: OH[c, e, b, n] = (dstloc[c, b*128+e] == n)
    dl = dstloc.reshape(NCORES, NBLK, P)
    OH = np.zeros((NCORES, 128, NBLK, P), f8e4)
    nn = np.arange(P, dtype=np.int64)
    for c in range(NCORES):
        oh_c = (dl[c][:, :, None] == nn[None, None, :])  # [b, e, n] bool
        OH[c] = np.ascontiguousarray(oh_c.transpose(1, 0, 2)).astype(f8e4)

    kW0 = np.asarray(inputs["kW0"], np.float32)
    kb0 = np.asarray(inputs["kb0"], np.float32)
    kW1 = np.asarray(inputs["kW1"], np.float32)
    vW0 = np.asarray(inputs["vW0"], np.float32)
    vb0 = np.asarray(inputs["vb0"], np.float32)
    vW1 = np.asarray(inputs["vW1"], np.float32)
    vb1 = np.asarray(inputs["vb1"], np.float32)
    q = np.asarray(inputs["q"], np.float32)

    qmask = np.zeros((D, H), np.float32)
    for h in range(H):
        qmask[h * DH:(h + 1) * DH, h] = q[0, h * DH:(h + 1) * DH] * SCALE

    wkfull = np.zeros((162, D), np.float32)
    wkfull[:160] = kW0
    weights = dict(
        W0K=wkfull.reshape(81, 2, D).astype(f8e4),
        kb0col=(SK * kb0).reshape(P, 1),
        W0vA=np.ascontiguousarray(vW0[:128]).astype(bf),
        W0vB=np.ascontiguousarray(vW0[128:160]).astype(bf),
        vb0col=(SV * vb0).reshape(P, 1),
        AQ8=((np.eye(D, dtype=np.float32) + kW1) @ qmask).astype(bf),
        MW1v=(np.eye(D, dtype=np.float32) + vW1).astype(bf),
        b1v_rep=np.tile(SV * vb1[None, :], (P, 1)),
        oW0=(np.asarray(inputs["oW0"], np.float32) / SV).astype(bf),
        ob0=np.asarray(inputs["ob0"], np.float32).reshape(P, 1),
        MoW1=(np.eye(D, dtype=np.float32)
              + np.asarray(inputs["oW1"], np.float32)).astype(bf),
        ob1=np.asarray(inputs["ob1"], np.float32).reshape(P, 1),
        IDENT=np.eye(P, dtype=np.float32),
    )
    use_b1v = bool(np.any(weights["b1v_rep"]))
    # single paired relu evac needs identical per-partition bias on both paths
    same_bias = bool(np.array_equal(weights["kb0col"], weights["vb0col"]))
    biases = dict(kb0=bool(np.any(kb0)), vb0=bool(np.any(vb0)),
                  ob0=bool(np.any(weights["ob0"])),
                  ob1=bool(np.any(weights["ob1"])))
    meta = dict(B=B, Bcum=Bcum, NBLK=NBLK, EPAD=EPAD, orders=orders,
                use_b1v=use_b1v, biases=biases, need_eps=need_eps,
                same_bias=same_bias)
    staged = dict(XA=XA, XB=XB, XK=XK, OH=OH)
    return staged, weights, meta


def _unshard(out_cores, orders):
    full = np.zeros((NT_TOTAL * P, D), np.float32)
    for c in range(NCORES):
        for j in range(TPC):
            t = int(orders[c, j])
            full[t * P:(t + 1) * P] = out_cores[c][:, j * P:(j + 1) * P].T
    return np.ascontiguousarray(full[:N])


# ------------------------------------------------------------- bass program
def build_program(B, Bcum, NBLK, EPAD, use_b1v, biases=None, need_eps=False,
                  same_bias=True, tpc=TPC, npc=None):
    biases = biases or {}
    npc = npc if npc is not None else tpc * P
    nc = bass.Bass("TRN2", target_bir_lowering=False, debug=False)
    XA_d = nc.declare_dram_parameter("XA", [128, EPAD], F8E3, isOutput=False)
    XB_d = nc.declare_dram_parameter("XB", [32, EPAD], F8E3, isOutput=False)
    XK_d = nc.declare_dram_parameter("XK", [81, 2, EPAD], F8E4, isOutput=False)
    OH_d = nc.declare_dram_parameter("OH", [128, NBLK, P], F8E4, isOutput=False)
    wnames = ["W0K", "kb0col", "W0vA", "W0vB", "vb0col", "AQ8",
              "MW1v", "b1v_rep", "oW0", "ob0", "MoW1", "ob1", "IDENT"]
    wshapes = {"W0K": [81, 2, 128], "kb0col": [128, 1],
               "W0vA": [128, 128], "W0vB": [32, 128], "vb0col": [128, 1],
               "AQ8": [128, 8], "MW1v": [128, 128], "b1v_rep": [128, 128],
               "oW0": [128, 128], "ob0": [128, 1], "MoW1": [128, 128],
               "ob1": [128, 1], "IDENT": [128, 128]}
    wdt = {"b1v_rep": F32, "ob0": F32, "ob1": F32, "kb0col": F32,
           "vb0col": F32, "W0K": F8E4, "IDENT": F32}
    w_d = {n: nc.declare_dram_parameter(n, wshapes[n], wdt.get(n, BF16),
                                        isOutput=False)
           for n in wnames}
    OUT_d = nc.declare_dram_parameter("OUT", [128, npc], F32, isOutput=True)

    SLAB = SLAB_BLOCKS * P
    assert NBLK % SLAB_BLOCKS == 0
    nslabs = NBLK // SLAB_BLOCKS
    NCH = NBLK // 4                 # 512-edge chunks overall

    with ExitStack() as ctx:
        tc = ctx.enter_context(tile.TileContext(nc))
        cpool = ctx.enter_context(tc.tile_pool(name="consts", bufs=1))
        xpool = ctx.enter_context(tc.tile_pool(name="x", bufs=3))
        ohpool = ctx.enter_context(tc.tile_pool(name="ohp", bufs=3))
        hkvpool = ctx.enter_context(tc.tile_pool(name="hkv", bufs=2))
        empool = ctx.enter_context(tc.tile_pool(name="em", bufs=2))
        npool = ctx.enter_context(tc.tile_pool(name="node", bufs=2))
        ps_l0 = ctx.enter_context(tc.tile_pool(name="psl0", bufs=1, space="PSUM"))
        ps_sc = ctx.enter_context(tc.tile_pool(name="pssc", bufs=1, space="PSUM"))
        ps_v = ctx.enter_context(tc.tile_pool(name="psv", bufs=1, space="PSUM"))
        ps_s = ctx.enter_context(tc.tile_pool(name="pss", bufs=2, space="PSUM"))
        ps_ep = ctx.enter_context(tc.tile_pool(name="psep", bufs=1, space="PSUM"))

        # --- persistent constants ---
        w_sb = {}
        for n in wnames:
            if n == "W0vB":
                t = cpool.tile([128, 128], BF16, name=f"w_{n}")
                nc.sync.dma_start(t[96:128, :], w_d[n][:])
            else:
                t = cpool.tile(wshapes[n], wdt.get(n, BF16), name=f"w_{n}")
                nc.sync.dma_start(t[:], w_d[n][:])
            w_sb[n] = t

        # --- slab tiles (created lazily, kept in dicts) ---
        xa_t = {}
        xb_t = {}
        xk_t = {}
        oh_t = {}
        hkv_t = {}
        exvs_t = {}

        def dma_slab(s):
            if s >= nslabs:
                return
            e0 = s * SLAB
            xa = xpool.tile([128, SLAB], F8E3, tag="xa", name=f"xa{s}")
            xb = xpool.tile([128, SLAB], F8E3, tag="xb", name=f"xb{s}")
            xk = xpool.tile([81, 2, SLAB], F8E4, tag="xk", name=f"xk{s}")
            oh = ohpool.tile([128, SLAB_BLOCKS, P], F8E4, tag="oh",
                             name=f"oh{s}")
            nc.sync.dma_start(xa[:, :], XA_d[:, e0:e0 + SLAB])
            nc.sync.dma_start(xb[96:128, :], XB_d[:, e0:e0 + SLAB])
            nc.sync.dma_start(xk[:, :, :], XK_d[:, :, e0:e0 + SLAB])
            nc.sync.dma_start(oh[:, :, :],
                              OH_d[:, s * SLAB_BLOCKS:(s + 1) * SLAB_BLOCKS, :])
            xa_t[s], xb_t[s], xk_t[s], oh_t[s] = xa, xb, xk, oh

        kb = w_sb["kb0col"][:] if biases.get("kb0") else 0.0
        vb = w_sb["vb0col"][:] if biases.get("vb0") else 0.0

        # --- pipeline state ---
        pending_octets = []     # octet indices whose exp has been emitted
        pending_epi = []        # deferred epilogue closures
        j_tile = [0]            # current node tile
        S_tile = [None]
        l0_pairs = {}           # chunk -> psum pair tile

        def emit_l0(p):
            s, c = divmod(p, 8)
            c0 = c * 512
            pair = ps_l0.tile([128, 2, 512], F32, tag="l0", name=f"l0_{p}")
            l0_pairs[p] = pair
            nc.tensor.matmul(pair[:, 0, :], w_sb["W0K"][:],
                             xk_t[s][:, :, c0:c0 + 512], start=True,
                             stop=True, skip_group_check=True,
                             perf_mode=mybir.MatmulPerfMode.DoubleRow)
            nc.tensor.matmul(pair[:, 1, :], w_sb["W0vB"][96:128, :],
                             xb_t[s][96:128, c0:c0 + 512], start=True,
                             stop=False, skip_group_check=True,
                             tile_position=(96, 0))
            nc.tensor.matmul(pair[:, 1, :], w_sb["W0vA"][:],
                             xa_t[s][:, c0:c0 + 512], start=False, stop=True,
                             skip_group_check=True)
            # paired relu evacuation into interleaved e3m4 hkv tile
            if s not in hkv_t:
                hkv_t[s] = hkvpool.tile([128, 8, 2, 512], F8E3, tag="hkv",
                                        name=f"hkv{s}")
            if same_bias:
                evac_dve = (c in EVAC_DVE_POS)
                if evac_dve:
                    nc.vector.tensor_scalar(
                        hkv_t[s][:, c, :, :], pair[:, :, :], SK, 0.0,
                        op0=mybir.AluOpType.mult, op1=mybir.AluOpType.max)
                else:
                    nc.scalar.activation(hkv_t[s][:, c, :, :], pair[:, :, :],
                                         mybir.ActivationFunctionType.Relu,
                                         bias=kb, scale=SK)
            else:
                nc.scalar.activation(hkv_t[s][:, c, 0, :], pair[:, 0, :],
                                     mybir.ActivationFunctionType.Relu,
                                     bias=kb, scale=SK)
                nc.scalar.activation(hkv_t[s][:, c, 1, :], pair[:, 1, :],
                                     mybir.ActivationFunctionType.Relu,
                                     bias=vb, scale=SV)

        sc_half = [None]
        v_oct = [None]

        def emit_em(g):
            """scores+v matmuls for global chunk g (4 blocks)."""
            s = g // 8
            hkv = hkv_t[s]
            c = g % 8
            for i in range(4):
                b = g * 4 + i           # global block
                bb = b % SLAB_BLOCKS    # block within slab
                if bb % 16 == 0:
                    sc_half[0] = ps_sc.tile([128, 16, 8], F32, tag="sc",
                                            name=f"sc{b // 16}")
                    sc_tiles[b // 16] = sc_half[0]
                if bb % 8 == 0:
                    v_oct[0] = ps_v.tile([128, 8, 128], F32, tag="voct",
                                         name=f"vo{b // 8}")
                hk_sl = hkv[:, c, 0, (bb % 4) * 128:(bb % 4) * 128 + 128]
                hv_sl = hkv[:, c, 1, (bb % 4) * 128:(bb % 4) * 128 + 128]
                nc.tensor.matmul(sc_half[0][:, bb % 16, :], hk_sl,
                                 w_sb["AQ8"][:], start=True, stop=True,
                                 skip_group_check=True)
                nc.tensor.matmul(v_oct[0][:, bb % 8, :], hv_sl,
                                 w_sb["MW1v"][:], start=True, stop=True,
                                 skip_group_check=True)
                if b % 8 == 7:
                    octs.setdefault(b // 8, []).append(v_oct[0])
            if use_b1v:
                nc.vector.tensor_tensor(
                    v_oct[0][:, (g % 2) * 4:(g % 2) * 4 + 4, :],
                    v_oct[0][:, (g % 2) * 4:(g % 2) * 4 + 4, :],
                    w_sb["b1v_rep"][:].unsqueeze(1).broadcast_to([128, 4, 128]),
                    op=mybir.AluOpType.add)

        octs = {}
        sc_tiles = {}

        def emit_exp(hf):
            """exp for half-slab hf (16 blocks) into exvs[:, :, 128:136]."""
            s = hf // 2
            h = hf % 2
            if s not in exvs_t:
                exvs_t[s] = empool.tile([128, SLAB_BLOCKS, 136], BF16,
                                        tag="exvs", name=f"exvs{s}")
            sc = sc_tiles.pop(hf)
            nc.scalar.activation(
                exvs_t[s][:, h * 16:h * 16 + 16, 128:136], sc[:, :, :],
                mybir.ActivationFunctionType.Exp, scale=1.0 / SK)

        def emit_epilogue_pe(jt, ep, gtile, out_col):
            def run():
                nc.tensor.transpose(ep[:, 0:128], gtile[:], w_sb["IDENT"][:])
                gfm = npool.tile([128, 128], BF16, tag="gfm", name=f"gfm{jt}")
                nc.scalar.activation(gfm[:], ep[:, 0:128],
                                     mybir.ActivationFunctionType.Relu)
                nc.tensor.matmul(ep[:, 128:256], w_sb["oW0"][:], gfm[:],
                                 start=True, stop=True, skip_group_check=True)
                h0 = npool.tile([128, 128], BF16, tag="h0", name=f"h0{jt}")
                nc.scalar.activation(
                    h0[:], ep[:, 128:256], mybir.ActivationFunctionType.Relu,
                    bias=w_sb["ob0"][:] if biases.get("ob0") else 0.0)
                nc.tensor.matmul(ep[:, 256:384], w_sb["MoW1"][:], h0[:],
                                 start=True, stop=True, skip_group_check=True)
                ot = npool.tile([128, 128], F32, tag="ot", name=f"ot{jt}")
                nc.scalar.activation(
                    ot[:], ep[:, 256:384], mybir.ActivationFunctionType.Relu,
                    bias=w_sb["ob1"][:] if biases.get("ob1") else 0.0)
                nc.sync.dma_start(OUT_d[:, out_col:out_col + 128], ot[:])
            return run

        def drain():
            # deferred epilogue PE chains first (their DVE deps are done)
            while pending_epi:
                pending_epi.pop(0)()
            while pending_octets:
                o = pending_octets.pop(0)
                s = o // 4
                vt = octs.pop(o)[0]
                exvs = exvs_t[s]
                o8 = (o % 4) * 8
                nc.vector.tensor_tensor(
                    exvs[:, o8:o8 + 8, 0:128].rearrange(
                        "p q (h r) -> p q h r", r=DH),
                    vt[:, :, :].rearrange("p q (h r) -> p q h r", r=DH),
                    exvs[:, o8:o8 + 8, 128:136].unsqueeze(3).broadcast_to(
                        [128, 8, 8, DH]),
                    op=mybir.AluOpType.mult)
                for i in range(8):
                    b = o * 8 + i
                    bb = b % SLAB_BLOCKS
                    jt = j_tile[0]
                    first = (b == Bcum[jt])
                    last = (b == Bcum[jt + 1] - 1)
                    if first:
                        S_tile[0] = ps_s.tile([128, 144], F32, tag="S",
                                              name=f"S{jt}")
                    nc.tensor.matmul(S_tile[0][:, 0:136], oh_t[s][:, bb, :],
                                     exvs[:, bb, :], start=first, stop=last,
                                     skip_group_check=True)
                    if last:
                        S = S_tile[0]
                        r1 = npool.tile([128, 8], F32, tag="r1",
                                        name=f"r1_{jt}")
                        if need_eps:
                            s1 = npool.tile([128, 8], F32, tag="s1",
                                            name=f"s1_{jt}")
                            nc.scalar.activation(
                                s1[:], S[:, 128:136],
                                mybir.ActivationFunctionType.Copy, bias=1e-30)
                            nc.vector.reciprocal(r1[:], s1[:])
                        else:
                            nc.vector.reciprocal(r1[:], S[:, 128:136])
                        g = npool.tile([128, 128], F32, tag="g", name=f"g{jt}")
                        nc.vector.tensor_tensor(
                            g[:].rearrange("p (h r) -> p h r", r=DH),
                            S[:, 0:128].rearrange("p (h r) -> p h r", r=DH),
                            r1[:].unsqueeze(2).broadcast_to([128, 8, DH]),
                            op=mybir.AluOpType.mult)
                        ep = ps_ep.tile([128, 384], F32, tag="ep",
                                        name=f"ep{jt}")
                        pending_epi.append(
                            emit_epilogue_pe(jt, ep, g, jt * 128))
                        j_tile[0] += 1

        # --- main emission loop ---
        dma_slab(0)
        dma_slab(1)
        for p in range(NCH + 3):
            s, c = divmod(p, 8)
            if p < NCH:
                if c == 0:
                    dma_slab(s + 2)
                emit_l0(p)
            g = p - 1
            if 0 <= g < NCH:
                emit_em(g)
                if g % 4 == 3:
                    hf = g // 4
                    emit_exp(hf)
                    pending_octets.extend([hf * 2, hf * 2 + 1])
            drain()
        while pending_epi:
            pending_epi.pop(0)()

        # register sc tiles for exp through a dict keyed by half index
        # (handled inline above)
    _split_excess_waits(nc)
    return nc


# ------------------------------------------------------------------ kernel
def kernel(**inputs):
    staged, weights, meta = _prep(inputs)
    nc = build_program(meta["B"], meta["Bcum"], meta["NBLK"], meta["EPAD"],
                       meta["use_b1v"], biases=meta["biases"],
                       need_eps=meta["need_eps"], same_bias=meta["same_bias"])
    in_maps = []
    for c in range(NCORES):
        m = {"XA": staged["XA"][c], "XB": staged["XB"][c],
             "XK": staged["XK"][c], "OH": staged["OH"][c]}
        m.update(weights)
        in_maps.append(m)
    res = run_bass_kernel_spmd(nc, in_maps, list(range(NCORES)))
    global LAST_EXEC_NS, LAST_RESULT
    LAST_EXEC_NS = getattr(res, "exec_time_ns", None)
    LAST_RESULT = res
    out_cores = [res.results[c]["OUT"] for c in range(NCORES)]
    return _unshard(out_cores, meta["orders"])
